# revision 42
# baseline (speedup 1.0000x reference)
"""Distributed KNN online evaluator kernel for 8 trn2 NeuronCores.

Device side (SPMD over 8 cores, bank sharded over N, zero padding,
fp8-e4m3 inputs):
  - fp8 matmul sim tiles (queries stationary) -> f32 PSUM,
    1024-col groups on a 4-deep PSUM ring (decouples PE from drain)
  - per-group blockmax-of-8 evacuation, balanced across engines and the
    shared DMA fabric (PSUM reads cost 1 f32/cycle/lane on DVE or ACT):
      B: DVE tensor_reduce straight from PSUM -> fp8 obuf (compacted)
      C: ACT copy psum -> sbuf fp8, raw sims DMA'd to HBM (host folds)
    (A1: ACT copy + DVE fold tree - available but unused in the mix)
  - Pool/SWDGE issues every outbound DMA (issuing from SP/ACT would
    stall their streams); folded blockmaxes leave in merged range DMAs

Host side:
  - assemble per-block bounds from folded blockmaxes + raw sims
  - adaptive drill-down: select blocks whose bound could contain a
    global top-K sim, recompute those sims exactly in f32, take top-K
  - class votes with inf weights degenerate to membership -> output is
    [voted classes asc, unvoted classes asc] per query
"""

import contextlib

import numpy as np
import ml_dtypes

import concourse.bass as bass
import concourse.mybir as mybir
from concourse.bass_utils import run_bass_kernel_spmd

BF16 = ml_dtypes.bfloat16
FP8 = ml_dtypes.float8_e4m3

N_CORES = 8
B = 256          # queries
D = 128          # feature dim
N_TOTAL = 200000
N_SHARD = N_TOTAL // N_CORES   # 25000, no padding
GROUP = 1024                   # columns per full psum group (2 banks)
N_FULL = 24                    # full groups per chunk
RUMP = N_SHARD - N_FULL * GROUP  # 424 (= 53 blocks of 8)
N_BG = N_FULL + 1              # 25 bank groups per chunk
N_STEPS = 2 * N_BG             # 50 (chunk, group) steps per iteration
PSUM_RING = 4                  # 4 x 1024 f32 = all 8 PSUM banks
BLK = 8
SPG = GROUP // BLK             # 128 slots per full group
SLOTS = N_SHARD // BLK         # 3125 per chunk
K = 200
NUM_CLASSES = 1000
MARGIN = 5.5   # fp8-input sim fuzz + fp8 output rounding, vs exact f32

BANK_RING = 4   # ring slots of 2*GROUP (one load covers 2 bank groups)
STAGE_RING = 6
MERGE_BG = 6   # folded groups per merged blockmax out DMA


def _gen_routes(n, counts):
    """Evenly interleave route classes over n slots (largest remainder)."""
    out = []
    used = {k: 0 for k in counts}
    for i in range(n):
        best, bestv = None, -1e9
        for k, c in counts.items():
            v = c * (i + 1) / n - used[k]
            if v > bestv:
                best, bestv = k, v
        used[best] += 1
        out.append(best)
    return out


# Route per full step (48 entries), rump steps (i=48,49) are always B.
ROUTE_FULL = _gen_routes(2 * N_FULL, {"B": 22, "C": 26})
assert len(ROUTE_FULL) == 2 * N_FULL


def _step_info(i):
    """Static per-step facts for step i in [0, N_STEPS)."""
    c, bg = i % 2, i // 2
    if bg < N_FULL:
        route = ROUTE_FULL[i]
        width = GROUP
        nslots = SPG
    else:
        route = "B"
        width = RUMP
        nslots = RUMP // BLK
    col0 = bg * GROUP
    return c, bg, route, width, nslots, col0


# raw-slot index per C step (per chunk), in step order
_C_STEPS = [i for i in range(2 * N_FULL) if ROUTE_FULL[i] == "C"]
N_RAW = max(len([i for i in _C_STEPS if i % 2 == c]) for c in (0, 1))
_RSLOT = {}
for _c in (0, 1):
    for _r, _i in enumerate([i for i in _C_STEPS if i % 2 == _c]):
        _RSLOT[_i] = _r

# compacted folded-slot layout: per chunk, folded (non-C) groups pack
# their block slots back to back in bg order; C groups get no obuf space
FSLOT = {}
NF = [0, 0]
for _c in (0, 1):
    _off = 0
    for _bg in range(N_BG):
        _i = 2 * _bg + _c
        _, _, _r, _w, _ns, _ = _step_info(_i)
        if _r != "C":
            FSLOT[_i] = _off
            _off += _ns
    NF[_c] = _off
NFMAX = max(NF)

# merged blockmax out units: (chunk, slot lo, slot hi, last contributing i)
_OUT_UNITS = []
for _c in (0, 1):
    _folded = [2 * _bg + _c for _bg in range(N_BG)
               if (2 * _bg + _c) in FSLOT]
    for _j in range(0, len(_folded), MERGE_BG):
        _grp = _folded[_j:_j + MERGE_BG]
        _lo = FSLOT[_grp[0]]
        _hi = FSLOT[_grp[-1]] + _step_info(_grp[-1])[4]
        _OUT_UNITS.append((_c, _lo, _hi, _grp[-1]))
N_UNITS = len(_OUT_UNITS) + len(_C_STEPS)


def _unit_of(i):
    """Out-DMA unit covering step i's output (for cross-repeat reuse)."""
    c = i % 2
    if i in _RSLOT:
        return len(_OUT_UNITS) + _C_STEPS.index(i)
    for u, (uc, lo, hi, _) in enumerate(_OUT_UNITS):
        if uc == c and lo <= FSLOT[i] < hi:
            return u
    raise AssertionError


_NC_CACHE = {}


def _build_nc(repeats=1):
    T = repeats * N_STEPS
    info = [_step_info(t % N_STEPS) for t in range(T)]
    route = [f[2] for f in info]
    # cumulative counts including index t
    nACT = np.cumsum([r in ("A1", "C") for r in route])
    nA1 = np.cumsum([r == "A1" for r in route])
    nB = np.cumsum([r == "B" for r in route])
    act_list = [t for t in range(T) if route[t] in ("A1", "C")]
    posACT = {t: k for k, t in enumerate(act_list)}  # 0-based among ACT steps

    nc = bass.Bass("TRN2", target_bir_lowering=False, debug=False,
                   num_devices=N_CORES)
    qT = nc.dram_tensor("qT", [D, B], mybir.dt.float8e4,
                        kind="ExternalInput").ap()
    bankT = nc.dram_tensor("bankT", [D, N_SHARD], mybir.dt.float8e4,
                           kind="ExternalInput").ap()
    out = nc.dram_tensor("blockmax", [B, NFMAX], mybir.dt.float8e4,
                         kind="ExternalOutput").ap()
    raw = nc.dram_tensor("rawsim", [B, N_RAW * GROUP], mybir.dt.float8e4,
                         kind="ExternalOutput").ap()

    MAX = mybir.AluOpType.max

    with contextlib.ExitStack() as ctx:
        qs = ctx.enter_context(nc.sbuf_tensor([D, B], mybir.dt.float8e4))
        banks = ctx.enter_context(
            nc.sbuf_tensor([D, BANK_RING * 2 * GROUP], mybir.dt.float8e4))
        psum = ctx.enter_context(
            nc.psum_tensor([128, PSUM_RING * GROUP], mybir.dt.float32))
        stage = ctx.enter_context(
            nc.sbuf_tensor([128, STAGE_RING * GROUP], mybir.dt.float8e4))
        l1 = ctx.enter_context(
            nc.sbuf_tensor([128, GROUP // 2], mybir.dt.bfloat16))
        l2 = ctx.enter_context(
            nc.sbuf_tensor([128, GROUP // 4], mybir.dt.bfloat16))
        obuf = ctx.enter_context(
            nc.sbuf_tensor([128, 2 * NFMAX], mybir.dt.float8e4))
        # order-robust DMA tracking: one sem per bank ring slot / out unit
        qsem = ctx.enter_context(nc.semaphore("qsem"))       # qT load, +16
        bank_sem = [ctx.enter_context(nc.semaphore(f"bank_sem{i}"))
                    for i in range(BANK_RING)]               # +16 per load
        osem = [ctx.enter_context(nc.semaphore(f"osem{i}"))
                for i in range(N_UNITS)]                     # +16 per repeat
        mm_sem = ctx.enter_context(nc.semaphore("mm_sem"))   # +1/step (PE)
        evacA = ctx.enter_context(nc.semaphore("evacA"))     # +1/ACT copy
        dvedone = ctx.enter_context(nc.semaphore("dvedone"))  # +1/DVE B
        dvefold = ctx.enter_context(nc.semaphore("dvefold"))  # +1/DVE A1
        block = ctx.enter_context(nc.Block())

        def psl(t, w):
            s = (t % PSUM_RING) * GROUP
            return psum[:, s:s + w]

        def oslot(t):
            c, bg, r, w, ns, col0 = info[t]
            f0 = FSLOT[t % N_STEPS]
            return obuf[:, c * NFMAX + f0:c * NFMAX + f0 + ns]

        @block.sync
        def _(sync):
            # one load = two bank groups (2048 cols; rump load is smaller)
            n_loads = repeats * ((N_BG + 1) // 2)
            lpr = (N_BG + 1) // 2  # loads per repeat

            def load_bank(ld):
                bg = (ld % lpr) * 2
                w = min(2 * GROUP, N_SHARD - bg * GROUP)
                sync.dma_start(
                    banks[:, (ld % BANK_RING) * 2 * GROUP:
                          (ld % BANK_RING) * 2 * GROUP + w],
                    bankT[:, bg * GROUP:bg * GROUP + w],
                ).then_inc(bank_sem[ld % BANK_RING], 16)

            for ld in range(min(BANK_RING, n_loads)):
                load_bank(ld)
            for ld in range(BANK_RING, n_loads):
                sync.wait_ge(mm_sem, 4 * (ld - BANK_RING) + 4)
                load_bank(ld)

        @block.gpsimd
        def _(gpsimd):
            # ALL DMAs other than inputs go via SWDGE on the otherwise-
            # idle Pool engine: issuing a DMA serializes with the issuing
            # engine's stream, so neither SP (bank prefetch) nor ACT
            # (psum drain) can afford them
            gpsimd.dma_start(qs[:], qT).then_inc(qsem, 16)
            for t in range(T):
                i = t % N_STEPS
                if route[t] == "C":  # raw stage slice out
                    gpsimd.wait_ge(evacA, posACT[t] + 1)
                    c = info[t][0]
                    ss = (posACT[t] % STAGE_RING) * GROUP
                    rs = _RSLOT[i] * GROUP
                    gpsimd.dma_start(
                        raw[c * 128:(c + 1) * 128, rs:rs + GROUP],
                        stage[:, ss:ss + GROUP],
                    ).then_inc(osem[_unit_of(i)], 16)
                for uo, (c, lo, hi, last_i) in enumerate(_OUT_UNITS):
                    if i != last_i:
                        continue
                    gpsimd.wait_ge(dvedone, nB[t])
                    if nA1[t]:
                        gpsimd.wait_ge(dvefold, nA1[t])
                    gpsimd.dma_start(
                        out[c * 128:(c + 1) * 128, lo:hi],
                        obuf[:, c * NFMAX + lo:c * NFMAX + hi],
                    ).then_inc(osem[uo], 16)

        @block.tensor
        def _(tensor):
            for t in range(T):
                c, bg, r, w, ns, col0 = info[t]
                u = t // 2
                ld = t // 4
                if t == 0:
                    tensor.wait_ge(qsem, 16)
                if t % 4 == 0:
                    tensor.wait_ge(bank_sem[ld % BANK_RING],
                                   16 * (ld // BANK_RING + 1))
                if t >= PSUM_RING:
                    tp = t - PSUM_RING
                    if route[tp] in ("A1", "C"):
                        tensor.wait_ge(evacA, nACT[tp])
                    else:
                        tensor.wait_ge(dvedone, nB[tp])
                s = (t % PSUM_RING) * GROUP
                bb = ((t // 4) % BANK_RING) * 2 * GROUP + (bg % 2) * GROUP
                nmm = (w + 511) // 512
                for k in range(nmm):
                    kw = min(512, w - k * 512)
                    mm = tensor.matmul(
                        psum[:, s + k * 512: s + k * 512 + kw],
                        lhsT=qs[:, c * 128:(c + 1) * 128],
                        rhs=banks[:, bb + k * 512: bb + k * 512 + kw],
                        start=True, stop=True)
                    if k == nmm - 1:
                        mm.then_inc(mm_sem, 1)

        @block.scalar
        def _(scalar):
            # dummy copy: loads the ACT function table during pipeline fill
            scalar.wait_ge(qsem, 16)
            scalar.copy(stage[:, :B], qs[:])
            scalar.drain()
            for idx, t in enumerate(act_list):
                if idx >= STAGE_RING:
                    occ = act_list[idx - STAGE_RING]
                    if route[occ] == "A1":
                        scalar.wait_ge(dvefold, nA1[occ])
                    else:  # C: raw out-DMA of occ frees the slot
                        scalar.wait_ge(osem[_unit_of(occ % N_STEPS)],
                                       16 * (occ // N_STEPS + 1))
                scalar.wait_ge(mm_sem, t + 1)
                ss = (idx % STAGE_RING) * GROUP
                scalar.copy(stage[:, ss:ss + GROUP],
                            psl(t, GROUP)).then_inc(evacA, 1)

        @block.vector
        def _(vector):
            # B-reduces free PSUM slots and are ready at matmul time;
            # A1 folds are ready only after ACT's copy. Emit folds late
            # so reduces never queue behind them.
            dve_ops = sorted(
                (t for t in range(T) if route[t] != "C"),
                key=lambda t: t if route[t] == "B" else t + 3.2)
            for t in dve_ops:
                r = route[t]
                if t >= N_STEPS:
                    vector.wait_ge(osem[_unit_of(t % N_STEPS)],
                                   16 * (t // N_STEPS))
                if r == "B":
                    vector.wait_ge(mm_sem, t + 1)
                    vector.tensor_reduce(
                        out=oslot(t),
                        in_=psl(t, info[t][3]).rearrange(
                            "p (b w) -> p b w", w=BLK),
                        axis=mybir.AxisListType.X,
                        op=MAX,
                    ).then_inc(dvedone, 1)
                else:  # A1
                    vector.wait_ge(evacA, posACT[t] + 1)
                    ss = (posACT[t] % STAGE_RING) * GROUP
                    h1, h2 = GROUP // 2, GROUP // 4
                    vector.tensor_tensor(
                        out=l1[:], in0=stage[:, ss:ss + h1],
                        in1=stage[:, ss + h1:ss + GROUP], op=MAX)
                    vector.drain()
                    vector.tensor_tensor(
                        out=l2[:], in0=l1[:, :h2], in1=l1[:, h2:], op=MAX)
                    vector.drain()
                    vector.tensor_tensor(
                        out=oslot(t), in0=l2[:, :h2 // 2],
                        in1=l2[:, h2 // 2:], op=MAX).then_inc(dvefold, 1)

    return nc


def _get_nc(repeats=1):
    if repeats not in _NC_CACHE:
        _NC_CACHE[repeats] = _build_nc(repeats)
    return _NC_CACHE[repeats]


def _prep_in_maps(query_feature, feature_bank):
    qT = np.ascontiguousarray(
        query_feature.astype(np.float32).T).astype(FP8)  # [128, 256]
    fb = np.asarray(feature_bank, dtype=np.float32)
    in_maps = []
    for i in range(N_CORES):
        shard = fb[i * N_SHARD:(i + 1) * N_SHARD]
        bt = np.ascontiguousarray(shard.T).astype(FP8)  # [128, 25000]
        in_maps.append({"qT": qT, "bankT": bt})
    return in_maps


def _chunk_layout(c):
    """Local col idx [SLOTS, BLK] for chunk c's compacted slot order:
    folded groups (bg order, per-route block pattern), then C groups
    (raw-slot order, contiguous-8)."""
    cols = np.empty((SLOTS, BLK), dtype=np.int64)
    k = np.arange(BLK)
    off = 0
    for bg in range(N_BG):
        i = 2 * bg + c
        _, _, r, w, ns, col0 = _step_info(i)
        if r == "C":
            continue
        j = np.arange(ns)
        if r == "A1":
            blk = j[:, None] + SPG * k[None, :]
        else:
            blk = BLK * j[:, None] + k[None, :]
        cols[off:off + ns] = col0 + blk
        off += ns
    assert off == NF[c]
    for i in [i for i in _C_STEPS if i % 2 == c]:
        _, _, r, w, ns, col0 = _step_info(i)
        j = np.arange(ns)
        cols[off:off + ns] = col0 + BLK * j[:, None] + k[None, :]
        off += ns
    assert off == SLOTS
    return cols


def _core_blockmax(bmx, rawx, c):
    """One core's per-block values for chunk c in compacted slot order.

    bmx: [256, NFMAX] f32, rawx: [256, N_RAW*GROUP] f32 -> [128, SLOTS]
    """
    rows = slice(c * 128, (c + 1) * 128)
    nraw_c = len([i for i in _C_STEPS if i % 2 == c])
    rb = rawx[rows, :nraw_c * GROUP].reshape(128, nraw_c * SPG, BLK)
    return np.concatenate([bmx[rows, :NF[c]], rb.max(axis=2)], axis=1)


def _run_device(query_feature, feature_bank, repeats=1, in_maps=None):
    if in_maps is None:
        in_maps = _prep_in_maps(query_feature, feature_bank)
    nc = _get_nc(repeats)
    res = run_bass_kernel_spmd(nc, in_maps, list(range(N_CORES)))
    bm = np.empty((N_CORES, B, SLOTS), dtype=np.float32)
    for core in range(N_CORES):
        bmx = res.results[core]["blockmax"].astype(np.float32)
        rawx = res.results[core]["rawsim"].astype(np.float32)
        for c in (0, 1):
            bm[core, c * 128:(c + 1) * 128] = _core_blockmax(bmx, rawx, c)
    return bm, res


def _host_topk(bm, query_feature, feature_bank, nsel=640):
    """bm: [8, 256, SLOTS] f32 per-block bounds (compacted order).
    Returns top-K indices [B, K] into the full bank, matching f32 jax
    top_k semantics."""
    q = np.asarray(query_feature, dtype=np.float32)
    fb = np.ascontiguousarray(np.asarray(feature_bank, dtype=np.float32))
    nblk = N_CORES * SLOTS
    # global block id (8 contiguous bank rows) per compacted slot
    gblk = np.empty((2, nblk), dtype=np.int64)
    for ch in range(2):
        srows = _chunk_layout(ch)  # [SLOTS, BLK] local cols
        assert (srows[:, 0] % BLK == 0).all()
        assert (srows == srows[:, :1] + np.arange(BLK)).all()
        for cidx in range(N_CORES):
            gblk[ch, cidx * SLOTS:(cidx + 1) * SLOTS] = (
                srows[:, 0] + cidx * N_SHARD) // BLK
    fbB = fb.reshape(N_TOTAL // BLK, BLK, D)
    bm_flat = bm.transpose(1, 0, 2).reshape(B, nblk)

    # partial descending order of block bounds (top-M is plenty; fall
    # back to a full sort only for queries that outgrow it)
    M = min(8192, nblk)
    part = np.argpartition(-bm_flat, M - 1, axis=1)[:, :M]
    pv = np.take_along_axis(bm_flat, part, axis=1)
    o_loc = np.argsort(-pv, axis=1)
    order = np.take_along_axis(part, o_loc, axis=1)
    sel_sorted = np.take_along_axis(bm_flat, order, axis=1)

    topk_idx = np.empty((B, K), dtype=np.int64)
    pending = np.arange(B)
    nb = nsel
    while len(pending):
        if nb > M and M < nblk:
            order_f = np.argsort(-bm_flat[pending], axis=1)
            order = np.zeros((B, nblk), dtype=np.int64)
            order[pending] = order_f
            sel_sorted = np.full((B, nblk), -np.inf, dtype=np.float32)
            sel_sorted[pending] = np.take_along_axis(
                bm_flat[pending], order_f, axis=1)
            M = nblk
        nb = min(nb, nblk)
        P = len(pending)
        gids = gblk[(pending // 128)[:, None], order[pending, :nb]]
        gids.sort(axis=1)  # sorted gather is ~2x faster, order is free
        sims = np.einsum("qbrd,qd->qbr", fbB[gids], q[pending],
                         optimize=True).reshape(P, -1)
        rows = (gids[:, :, None] * BLK + np.arange(BLK)).reshape(P, -1)
        o = np.lexsort((rows, -sims), axis=-1)[:, :K]
        tK = sims[np.arange(P), o[:, -1]]
        unsel = (sel_sorted[pending, nb] if nb < nblk
                 else np.full(P, -np.inf, dtype=np.float32))
        done = (unsel + MARGIN < tK) | (nb >= nblk)
        sel = np.take_along_axis(rows, o, axis=1)
        topk_idx[pending[done]] = sel[done]
        pending = pending[~done]
        nb *= 2
    return topk_idx


def _labels_to_output(topk_idx, target_bank):
    tb = np.asarray(target_bank).astype(np.int64)
    lab = tb[topk_idx]  # [B, K]
    mask = np.zeros((B, NUM_CLASSES), dtype=bool)
    mask[np.arange(B)[:, None], lab] = True
    # inf vote weights -> membership only: voted classes (ascending) first,
    # then unvoted (ascending); matches stable argsort of -scores.
    return np.argsort(~mask, axis=1, kind="stable").astype(np.int32)


def kernel(query_feature, feature_bank, target_bank):
    query_feature = np.asarray(query_feature)
    feature_bank = np.asarray(feature_bank)
    target_bank = np.asarray(target_bank)
    bm, _ = _run_device(query_feature, feature_bank)
    topk_idx = _host_topk(bm, query_feature, feature_bank)
    return _labels_to_output(topk_idx, target_bank)


# revision 45
# speedup vs baseline: 1.0272x; 1.0272x over previous
"""Distributed KNN online evaluator kernel for 8 trn2 NeuronCores.

Device side (SPMD over 8 cores, bank sharded over N, zero padding,
fp8-e4m3 inputs):
  - fp8 matmul sim tiles (queries stationary) -> f32 PSUM,
    1024-col groups on a 4-deep PSUM ring (decouples PE from drain)
  - per-group blockmax-of-8 evacuation, balanced across engines and the
    shared DMA fabric (PSUM reads cost 1 f32/cycle/lane on DVE or ACT):
      B: DVE tensor_reduce straight from PSUM -> fp8 obuf (compacted)
      C: ACT copy psum -> sbuf fp8, raw sims DMA'd to HBM (host folds)
    (A1: ACT copy + DVE fold tree - available but unused in the mix)
  - Pool/SWDGE issues every outbound DMA (issuing from SP/ACT would
    stall their streams); folded blockmaxes leave in merged range DMAs

Host side:
  - assemble per-block bounds from folded blockmaxes + raw sims
  - adaptive drill-down: select blocks whose bound could contain a
    global top-K sim, recompute those sims exactly in f32, take top-K
  - class votes with inf weights degenerate to membership -> output is
    [voted classes asc, unvoted classes asc] per query
"""

import contextlib

import numpy as np
import ml_dtypes

import concourse.bass as bass
import concourse.mybir as mybir
from concourse.bass_utils import run_bass_kernel_spmd

BF16 = ml_dtypes.bfloat16
FP8 = ml_dtypes.float8_e4m3

N_CORES = 8
B = 256          # queries
D = 128          # feature dim
N_TOTAL = 200000
N_SHARD = N_TOTAL // N_CORES   # 25000, no padding
GROUP = 1024                   # columns per full psum group (2 banks)
N_FULL = 24                    # full groups per chunk
RUMP = N_SHARD - N_FULL * GROUP  # 424 (= 53 blocks of 8)
N_BG = N_FULL + 1              # 25 bank groups per chunk
N_STEPS = 2 * N_BG             # 50 (chunk, group) steps per iteration
PSUM_RING = 4                  # 4 x 1024 f32 = all 8 PSUM banks
BLK = 8
SPG = GROUP // BLK             # 128 slots per full group
SLOTS = N_SHARD // BLK         # 3125 per chunk
K = 200
NUM_CLASSES = 1000
MARGIN = 5.5   # fp8-input sim fuzz + fp8 output rounding, vs exact f32

BANK_RING = 4   # ring slots of 2*GROUP (one load covers 2 bank groups)
STAGE_RING = 6
MERGE_BG = 6   # folded groups per merged blockmax out DMA


def _gen_routes(n, counts):
    """Evenly interleave route classes over n slots (largest remainder)."""
    out = []
    used = {k: 0 for k in counts}
    for i in range(n):
        best, bestv = None, -1e9
        for k, c in counts.items():
            v = c * (i + 1) / n - used[k]
            if v > bestv:
                best, bestv = k, v
        used[best] += 1
        out.append(best)
    return out


# Route per full step (48 entries), rump steps (i=48,49) are always B.
ROUTE_FULL = _gen_routes(2 * N_FULL, {"B": 22, "C": 26})
assert len(ROUTE_FULL) == 2 * N_FULL


def _step_info(i):
    """Static per-step facts for step i in [0, N_STEPS)."""
    c, bg = i % 2, i // 2
    if bg < N_FULL:
        route = ROUTE_FULL[i]
        width = GROUP
        nslots = SPG
    else:
        route = "B"
        width = RUMP
        nslots = RUMP // BLK
    col0 = bg * GROUP
    return c, bg, route, width, nslots, col0


# raw-slot index per C step (per chunk), in step order
_C_STEPS = [i for i in range(2 * N_FULL) if ROUTE_FULL[i] == "C"]
N_RAW = max(len([i for i in _C_STEPS if i % 2 == c]) for c in (0, 1))
_RSLOT = {}
for _c in (0, 1):
    for _r, _i in enumerate([i for i in _C_STEPS if i % 2 == _c]):
        _RSLOT[_i] = _r

# compacted folded-slot layout: per chunk, folded (non-C) groups pack
# their block slots back to back in bg order; C groups get no obuf space
FSLOT = {}
NF = [0, 0]
for _c in (0, 1):
    _off = 0
    for _bg in range(N_BG):
        _i = 2 * _bg + _c
        _, _, _r, _w, _ns, _ = _step_info(_i)
        if _r != "C":
            FSLOT[_i] = _off
            _off += _ns
    NF[_c] = _off
NFMAX = max(NF)

# merged blockmax out units: (chunk, slot lo, slot hi, last contributing i)
_OUT_UNITS = []
for _c in (0, 1):
    _folded = [2 * _bg + _c for _bg in range(N_BG)
               if (2 * _bg + _c) in FSLOT]
    for _j in range(0, len(_folded), MERGE_BG):
        _grp = _folded[_j:_j + MERGE_BG]
        _lo = FSLOT[_grp[0]]
        _hi = FSLOT[_grp[-1]] + _step_info(_grp[-1])[4]
        _OUT_UNITS.append((_c, _lo, _hi, _grp[-1]))
N_UNITS = len(_OUT_UNITS) + len(_C_STEPS)


def _unit_of(i):
    """Out-DMA unit covering step i's output (for cross-repeat reuse)."""
    c = i % 2
    if i in _RSLOT:
        return len(_OUT_UNITS) + _C_STEPS.index(i)
    for u, (uc, lo, hi, _) in enumerate(_OUT_UNITS):
        if uc == c and lo <= FSLOT[i] < hi:
            return u
    raise AssertionError


_NC_CACHE = {}


def _build_nc(repeats=1):
    T = repeats * N_STEPS
    info = [_step_info(t % N_STEPS) for t in range(T)]
    route = [f[2] for f in info]
    # cumulative counts including index t
    nACT = np.cumsum([r in ("A1", "C") for r in route])
    nA1 = np.cumsum([r == "A1" for r in route])
    nB = np.cumsum([r == "B" for r in route])
    act_list = [t for t in range(T) if route[t] in ("A1", "C")]
    posACT = {t: k for k, t in enumerate(act_list)}  # 0-based among ACT steps

    nc = bass.Bass("TRN2", target_bir_lowering=False, debug=False,
                   num_devices=N_CORES)
    qT = nc.dram_tensor("qT", [D, B], mybir.dt.float8e4,
                        kind="ExternalInput").ap()
    bankT = nc.dram_tensor("bankT", [D, N_SHARD], mybir.dt.float8e4,
                           kind="ExternalInput").ap()
    out = nc.dram_tensor("blockmax", [B, NFMAX], mybir.dt.float8e4,
                         kind="ExternalOutput").ap()
    raw = nc.dram_tensor("rawsim", [B, N_RAW * GROUP], mybir.dt.float8e4,
                         kind="ExternalOutput").ap()

    MAX = mybir.AluOpType.max

    with contextlib.ExitStack() as ctx:
        qs = ctx.enter_context(nc.sbuf_tensor([D, B], mybir.dt.float8e4))
        banks = ctx.enter_context(
            nc.sbuf_tensor([D, BANK_RING * 2 * GROUP], mybir.dt.float8e4))
        psum = ctx.enter_context(
            nc.psum_tensor([128, PSUM_RING * GROUP], mybir.dt.float32))
        stage = ctx.enter_context(
            nc.sbuf_tensor([128, STAGE_RING * GROUP], mybir.dt.float8e4))
        l1 = ctx.enter_context(
            nc.sbuf_tensor([128, GROUP // 2], mybir.dt.bfloat16))
        l2 = ctx.enter_context(
            nc.sbuf_tensor([128, GROUP // 4], mybir.dt.bfloat16))
        obuf = ctx.enter_context(
            nc.sbuf_tensor([128, 2 * NFMAX], mybir.dt.float8e4))
        # order-robust DMA tracking: one sem per bank ring slot / out unit
        qsem = ctx.enter_context(nc.semaphore("qsem"))       # qT load, +16
        bank_sem = [ctx.enter_context(nc.semaphore(f"bank_sem{i}"))
                    for i in range(BANK_RING)]               # +16 per load
        osem = [ctx.enter_context(nc.semaphore(f"osem{i}"))
                for i in range(N_UNITS)]                     # +16 per repeat
        mm_sem = ctx.enter_context(nc.semaphore("mm_sem"))   # +1/step (PE)
        evacA = ctx.enter_context(nc.semaphore("evacA"))     # +1/ACT copy
        dvedone = ctx.enter_context(nc.semaphore("dvedone"))  # +1/DVE B
        dvefold = ctx.enter_context(nc.semaphore("dvefold"))  # +1/DVE A1
        warmsem = ctx.enter_context(nc.semaphore("warmsem"))  # ACT warm
        b0sem = ctx.enter_context(nc.semaphore("b0sem"))  # first half-load
        block = ctx.enter_context(nc.Block())

        def psl(t, w):
            s = (t % PSUM_RING) * GROUP
            return psum[:, s:s + w]

        def oslot(t):
            c, bg, r, w, ns, col0 = info[t]
            f0 = FSLOT[t % N_STEPS]
            return obuf[:, c * NFMAX + f0:c * NFMAX + f0 + ns]

        @block.sync
        def _(sync):
            # one load = two bank groups (2048 cols; rump load is
            # smaller). The very first load is split in two 1024 halves
            # so the first matmul starts ~0.5us earlier.
            n_loads = repeats * ((N_BG + 1) // 2)
            lpr = (N_BG + 1) // 2  # loads per repeat

            def load_bank(ld, half=None):
                bg = (ld % lpr) * 2
                base = (ld % BANK_RING) * 2 * GROUP
                w = min(2 * GROUP, N_SHARD - bg * GROUP)
                sem = bank_sem[ld % BANK_RING]
                if half is not None:  # ld == 0 split
                    bg += half
                    base += half * GROUP
                    w = GROUP
                    if half == 0:
                        sem = b0sem
                sync.dma_start(
                    banks[:, base:base + w],
                    bankT[:, bg * GROUP:bg * GROUP + w],
                ).then_inc(sem, 16)

            load_bank(0, half=0)
            load_bank(0, half=1)
            for ld in range(1, min(BANK_RING, n_loads)):
                load_bank(ld)
            for ld in range(BANK_RING, n_loads):
                sync.wait_ge(mm_sem, 4 * (ld - BANK_RING) + 4)
                load_bank(ld)

        @block.gpsimd
        def _(gpsimd):
            # ALL DMAs other than inputs go via SWDGE on the otherwise-
            # idle Pool engine: issuing a DMA serializes with the issuing
            # engine's stream, so neither SP (bank prefetch) nor ACT
            # (psum drain) can afford them
            gpsimd.dma_start(qs[:], qT).then_inc(qsem, 16)
            for t in range(T):
                i = t % N_STEPS
                if route[t] == "C":  # raw stage slice out
                    gpsimd.wait_ge(evacA, posACT[t] + 1)
                    c = info[t][0]
                    ss = (posACT[t] % STAGE_RING) * GROUP
                    rs = _RSLOT[i] * GROUP
                    gpsimd.dma_start(
                        raw[c * 128:(c + 1) * 128, rs:rs + GROUP],
                        stage[:, ss:ss + GROUP],
                    ).then_inc(osem[_unit_of(i)], 16)
                for uo, (c, lo, hi, last_i) in enumerate(_OUT_UNITS):
                    if i != last_i:
                        continue
                    gpsimd.wait_ge(dvedone, nB[t])
                    if nA1[t]:
                        gpsimd.wait_ge(dvefold, nA1[t])
                    gpsimd.dma_start(
                        out[c * 128:(c + 1) * 128, lo:hi],
                        obuf[:, c * NFMAX + lo:c * NFMAX + hi],
                    ).then_inc(osem[uo], 16)

        @block.tensor
        def _(tensor):
            for t in range(T):
                c, bg, r, w, ns, col0 = info[t]
                u = t // 2
                ld = t // 4
                if t == 0:
                    tensor.wait_ge(qsem, 16)
                if t == 0:
                    tensor.wait_ge(b0sem, 16)         # first 1024 half
                elif t == 2:
                    tensor.wait_ge(bank_sem[0], 16)   # second half
                elif t % 4 == 0:
                    tensor.wait_ge(bank_sem[ld % BANK_RING],
                                   16 * (ld // BANK_RING + 1))
                if t >= PSUM_RING:
                    tp = t - PSUM_RING
                    if route[tp] in ("A1", "C"):
                        tensor.wait_ge(evacA, nACT[tp])
                    else:
                        tensor.wait_ge(dvedone, nB[tp])
                s = (t % PSUM_RING) * GROUP
                bb = ((t // 4) % BANK_RING) * 2 * GROUP + (bg % 2) * GROUP
                nmm = (w + 511) // 512
                for k in range(nmm):
                    kw = min(512, w - k * 512)
                    mm = tensor.matmul(
                        psum[:, s + k * 512: s + k * 512 + kw],
                        lhsT=qs[:, c * 128:(c + 1) * 128],
                        rhs=banks[:, bb + k * 512: bb + k * 512 + kw],
                        start=True, stop=True)
                    if k == nmm - 1:
                        mm.then_inc(mm_sem, 1)

        @block.scalar
        def _(scalar):
            # dummy copy at t~0: pulls the ACT function-table load fully
            # into the pipeline-fill phase (seeded by DVE's memset, no
            # input-DMA dependency)
            scalar.wait_ge(warmsem, 1)
            scalar.copy(stage[:, 2:4], stage[:, :2])
            scalar.drain()
            for idx, t in enumerate(act_list):
                if idx >= STAGE_RING:
                    occ = act_list[idx - STAGE_RING]
                    if route[occ] == "A1":
                        scalar.wait_ge(dvefold, nA1[occ])
                    else:  # C: raw out-DMA of occ frees the slot
                        scalar.wait_ge(osem[_unit_of(occ % N_STEPS)],
                                       16 * (occ // N_STEPS + 1))
                scalar.wait_ge(mm_sem, t + 1)
                ss = (idx % STAGE_RING) * GROUP
                scalar.copy(stage[:, ss:ss + GROUP],
                            psl(t, GROUP)).then_inc(evacA, 1)

        @block.vector
        def _(vector):
            vector.memset(stage[:, :2], 0.0).then_inc(warmsem, 1)
            vector.drain()
            # B-reduces free PSUM slots and are ready at matmul time;
            # A1 folds are ready only after ACT's copy. Emit folds late
            # so reduces never queue behind them.
            dve_ops = sorted(
                (t for t in range(T) if route[t] != "C"),
                key=lambda t: t if route[t] == "B" else t + 3.2)
            for t in dve_ops:
                r = route[t]
                if t >= N_STEPS:
                    vector.wait_ge(osem[_unit_of(t % N_STEPS)],
                                   16 * (t // N_STEPS))
                if r == "B":
                    vector.wait_ge(mm_sem, t + 1)
                    vector.tensor_reduce(
                        out=oslot(t),
                        in_=psl(t, info[t][3]).rearrange(
                            "p (b w) -> p b w", w=BLK),
                        axis=mybir.AxisListType.X,
                        op=MAX,
                    ).then_inc(dvedone, 1)
                else:  # A1
                    vector.wait_ge(evacA, posACT[t] + 1)
                    ss = (posACT[t] % STAGE_RING) * GROUP
                    h1, h2 = GROUP // 2, GROUP // 4
                    vector.tensor_tensor(
                        out=l1[:], in0=stage[:, ss:ss + h1],
                        in1=stage[:, ss + h1:ss + GROUP], op=MAX)
                    vector.drain()
                    vector.tensor_tensor(
                        out=l2[:], in0=l1[:, :h2], in1=l1[:, h2:], op=MAX)
                    vector.drain()
                    vector.tensor_tensor(
                        out=oslot(t), in0=l2[:, :h2 // 2],
                        in1=l2[:, h2 // 2:], op=MAX).then_inc(dvefold, 1)

    return nc


def _get_nc(repeats=1):
    if repeats not in _NC_CACHE:
        _NC_CACHE[repeats] = _build_nc(repeats)
    return _NC_CACHE[repeats]


def _prep_in_maps(query_feature, feature_bank):
    qT = np.ascontiguousarray(
        query_feature.astype(np.float32).T).astype(FP8)  # [128, 256]
    fb = np.asarray(feature_bank, dtype=np.float32)
    in_maps = []
    for i in range(N_CORES):
        shard = fb[i * N_SHARD:(i + 1) * N_SHARD]
        bt = np.ascontiguousarray(shard.T).astype(FP8)  # [128, 25000]
        in_maps.append({"qT": qT, "bankT": bt})
    return in_maps


def _chunk_layout(c):
    """Local col idx [SLOTS, BLK] for chunk c's compacted slot order:
    folded groups (bg order, per-route block pattern), then C groups
    (raw-slot order, contiguous-8)."""
    cols = np.empty((SLOTS, BLK), dtype=np.int64)
    k = np.arange(BLK)
    off = 0
    for bg in range(N_BG):
        i = 2 * bg + c
        _, _, r, w, ns, col0 = _step_info(i)
        if r == "C":
            continue
        j = np.arange(ns)
        if r == "A1":
            blk = j[:, None] + SPG * k[None, :]
        else:
            blk = BLK * j[:, None] + k[None, :]
        cols[off:off + ns] = col0 + blk
        off += ns
    assert off == NF[c]
    for i in [i for i in _C_STEPS if i % 2 == c]:
        _, _, r, w, ns, col0 = _step_info(i)
        j = np.arange(ns)
        cols[off:off + ns] = col0 + BLK * j[:, None] + k[None, :]
        off += ns
    assert off == SLOTS
    return cols


def _core_blockmax(bmx, rawx, c):
    """One core's per-block values for chunk c in compacted slot order.

    bmx: [256, NFMAX] f32, rawx: [256, N_RAW*GROUP] f32 -> [128, SLOTS]
    """
    rows = slice(c * 128, (c + 1) * 128)
    nraw_c = len([i for i in _C_STEPS if i % 2 == c])
    rb = rawx[rows, :nraw_c * GROUP].reshape(128, nraw_c * SPG, BLK)
    return np.concatenate([bmx[rows, :NF[c]], rb.max(axis=2)], axis=1)


def _run_device(query_feature, feature_bank, repeats=1, in_maps=None):
    if in_maps is None:
        in_maps = _prep_in_maps(query_feature, feature_bank)
    nc = _get_nc(repeats)
    res = run_bass_kernel_spmd(nc, in_maps, list(range(N_CORES)))
    bm = np.empty((N_CORES, B, SLOTS), dtype=np.float32)
    for core in range(N_CORES):
        bmx = res.results[core]["blockmax"].astype(np.float32)
        rawx = res.results[core]["rawsim"].astype(np.float32)
        for c in (0, 1):
            bm[core, c * 128:(c + 1) * 128] = _core_blockmax(bmx, rawx, c)
    return bm, res


def _host_topk(bm, query_feature, feature_bank, nsel=640):
    """bm: [8, 256, SLOTS] f32 per-block bounds (compacted order).
    Returns top-K indices [B, K] into the full bank, matching f32 jax
    top_k semantics."""
    q = np.asarray(query_feature, dtype=np.float32)
    fb = np.ascontiguousarray(np.asarray(feature_bank, dtype=np.float32))
    nblk = N_CORES * SLOTS
    # global block id (8 contiguous bank rows) per compacted slot
    gblk = np.empty((2, nblk), dtype=np.int64)
    for ch in range(2):
        srows = _chunk_layout(ch)  # [SLOTS, BLK] local cols
        assert (srows[:, 0] % BLK == 0).all()
        assert (srows == srows[:, :1] + np.arange(BLK)).all()
        for cidx in range(N_CORES):
            gblk[ch, cidx * SLOTS:(cidx + 1) * SLOTS] = (
                srows[:, 0] + cidx * N_SHARD) // BLK
    fbB = fb.reshape(N_TOTAL // BLK, BLK, D)
    bm_flat = bm.transpose(1, 0, 2).reshape(B, nblk)

    # partial descending order of block bounds (top-M is plenty; fall
    # back to a full sort only for queries that outgrow it)
    M = min(8192, nblk)
    part = np.argpartition(-bm_flat, M - 1, axis=1)[:, :M]
    pv = np.take_along_axis(bm_flat, part, axis=1)
    o_loc = np.argsort(-pv, axis=1)
    order = np.take_along_axis(part, o_loc, axis=1)
    sel_sorted = np.take_along_axis(bm_flat, order, axis=1)

    topk_idx = np.empty((B, K), dtype=np.int64)
    pending = np.arange(B)
    nb = nsel
    while len(pending):
        if nb > M and M < nblk:
            order_f = np.argsort(-bm_flat[pending], axis=1)
            order = np.zeros((B, nblk), dtype=np.int64)
            order[pending] = order_f
            sel_sorted = np.full((B, nblk), -np.inf, dtype=np.float32)
            sel_sorted[pending] = np.take_along_axis(
                bm_flat[pending], order_f, axis=1)
            M = nblk
        nb = min(nb, nblk)
        P = len(pending)
        gids = gblk[(pending // 128)[:, None], order[pending, :nb]]
        gids.sort(axis=1)  # sorted gather is ~2x faster, order is free
        sims = np.einsum("qbrd,qd->qbr", fbB[gids], q[pending],
                         optimize=True).reshape(P, -1)
        rows = (gids[:, :, None] * BLK + np.arange(BLK)).reshape(P, -1)
        o = np.lexsort((rows, -sims), axis=-1)[:, :K]
        tK = sims[np.arange(P), o[:, -1]]
        unsel = (sel_sorted[pending, nb] if nb < nblk
                 else np.full(P, -np.inf, dtype=np.float32))
        done = (unsel + MARGIN < tK) | (nb >= nblk)
        sel = np.take_along_axis(rows, o, axis=1)
        topk_idx[pending[done]] = sel[done]
        pending = pending[~done]
        nb *= 2
    return topk_idx


def _labels_to_output(topk_idx, target_bank):
    tb = np.asarray(target_bank).astype(np.int64)
    lab = tb[topk_idx]  # [B, K]
    mask = np.zeros((B, NUM_CLASSES), dtype=bool)
    mask[np.arange(B)[:, None], lab] = True
    # inf vote weights -> membership only: voted classes (ascending) first,
    # then unvoted (ascending); matches stable argsort of -scores.
    return np.argsort(~mask, axis=1, kind="stable").astype(np.int32)


def kernel(query_feature, feature_bank, target_bank):
    query_feature = np.asarray(query_feature)
    feature_bank = np.asarray(feature_bank)
    target_bank = np.asarray(target_bank)
    bm, _ = _run_device(query_feature, feature_bank)
    topk_idx = _host_topk(bm, query_feature, feature_bank)
    return _labels_to_output(topk_idx, target_bank)


# revision 47
# speedup vs baseline: 1.0405x; 1.0130x over previous
"""Distributed KNN online evaluator kernel for 8 trn2 NeuronCores.

Device side (SPMD over 8 cores, bank sharded over N, zero padding,
fp8-e4m3 inputs):
  - fp8 matmul sim tiles (queries stationary) -> f32 PSUM,
    1024-col groups on a 4-deep PSUM ring (decouples PE from drain)
  - per-group blockmax-of-8 evacuation, balanced across engines and the
    shared DMA fabric (PSUM reads cost 1 f32/cycle/lane on DVE or ACT):
      B: DVE tensor_reduce straight from PSUM -> fp8 obuf (compacted)
      C: ACT copy psum -> sbuf fp8, raw sims DMA'd to HBM (host folds)
    (A1: ACT copy + DVE fold tree - available but unused in the mix)
  - Pool/SWDGE issues every outbound DMA (issuing from SP/ACT would
    stall their streams); folded blockmaxes leave in merged range DMAs

Host side:
  - assemble per-block bounds from folded blockmaxes + raw sims
  - adaptive drill-down: select blocks whose bound could contain a
    global top-K sim, recompute those sims exactly in f32, take top-K
  - class votes with inf weights degenerate to membership -> output is
    [voted classes asc, unvoted classes asc] per query
"""

import contextlib

import numpy as np
import ml_dtypes

import concourse.bass as bass
import concourse.mybir as mybir
from concourse.bass_utils import run_bass_kernel_spmd

BF16 = ml_dtypes.bfloat16
FP8 = ml_dtypes.float8_e4m3

N_CORES = 8
B = 256          # queries
D = 128          # feature dim
N_TOTAL = 200000
N_SHARD = N_TOTAL // N_CORES   # 25000, no padding
GROUP = 1024                   # columns per full psum group (2 banks)
N_FULL = 24                    # full groups per chunk
RUMP = N_SHARD - N_FULL * GROUP  # 424 (= 53 blocks of 8)
N_BG = N_FULL + 1              # 25 bank groups per chunk
N_STEPS = 2 * N_BG             # 50 (chunk, group) steps per iteration
PSUM_RING = 4                  # 4 x 1024 f32 = all 8 PSUM banks
BLK = 8
SPG = GROUP // BLK             # 128 slots per full group
SLOTS = N_SHARD // BLK         # 3125 per chunk
K = 200
NUM_CLASSES = 1000
MARGIN = 5.5   # fp8-input sim fuzz + fp8 output rounding, vs exact f32

BANK_RING = 4   # ring slots of 2*GROUP (one load covers 2 bank groups)
STAGE_RING = 6
MERGE_BG = 6   # folded groups per merged blockmax out DMA


def _gen_routes(n, counts):
    """Evenly interleave route classes over n slots (largest remainder)."""
    out = []
    used = {k: 0 for k in counts}
    for i in range(n):
        best, bestv = None, -1e9
        for k, c in counts.items():
            v = c * (i + 1) / n - used[k]
            if v > bestv:
                best, bestv = k, v
        used[best] += 1
        out.append(best)
    return out


# Route per full step (48 entries), rump steps (i=48,49) are always B.
ROUTE_FULL = _gen_routes(2 * N_FULL, {"B": 22, "C": 26})
assert len(ROUTE_FULL) == 2 * N_FULL


def _step_info(i):
    """Static per-step facts for step i in [0, N_STEPS)."""
    c, bg = i % 2, i // 2
    if bg < N_FULL:
        route = ROUTE_FULL[i]
        width = GROUP
        nslots = SPG
    else:
        route = "B"
        width = RUMP
        nslots = RUMP // BLK
    col0 = bg * GROUP
    return c, bg, route, width, nslots, col0


# raw-slot index per C step (per chunk), in step order
_C_STEPS = [i for i in range(2 * N_FULL) if ROUTE_FULL[i] == "C"]
N_RAW = max(len([i for i in _C_STEPS if i % 2 == c]) for c in (0, 1))
_RSLOT = {}
for _c in (0, 1):
    for _r, _i in enumerate([i for i in _C_STEPS if i % 2 == _c]):
        _RSLOT[_i] = _r

# compacted folded-slot layout: per chunk, folded (non-C) groups pack
# their block slots back to back in bg order; C groups get no obuf space
FSLOT = {}
NF = [0, 0]
for _c in (0, 1):
    _off = 0
    for _bg in range(N_BG):
        _i = 2 * _bg + _c
        _, _, _r, _w, _ns, _ = _step_info(_i)
        if _r != "C":
            FSLOT[_i] = _off
            _off += _ns
    NF[_c] = _off
NFMAX = max(NF)

# merged blockmax out units: (chunk, slot lo, slot hi, last contributing i)
_OUT_UNITS = []
for _c in (0, 1):
    _folded = [2 * _bg + _c for _bg in range(N_BG)
               if (2 * _bg + _c) in FSLOT]
    for _j in range(0, len(_folded), MERGE_BG):
        _grp = _folded[_j:_j + MERGE_BG]
        _lo = FSLOT[_grp[0]]
        _hi = FSLOT[_grp[-1]] + _step_info(_grp[-1])[4]
        _OUT_UNITS.append((_c, _lo, _hi, _grp[-1]))
N_UNITS = len(_OUT_UNITS) + len(_C_STEPS)


def _unit_of(i):
    """Out-DMA unit covering step i's output (for cross-repeat reuse)."""
    c = i % 2
    if i in _RSLOT:
        return len(_OUT_UNITS) + _C_STEPS.index(i)
    for u, (uc, lo, hi, _) in enumerate(_OUT_UNITS):
        if uc == c and lo <= FSLOT[i] < hi:
            return u
    raise AssertionError


_NC_CACHE = {}


def _build_nc(repeats=1):
    T = repeats * N_STEPS
    info = [_step_info(t % N_STEPS) for t in range(T)]
    route = [f[2] for f in info]
    # cumulative counts including index t
    nACT = np.cumsum([r in ("A1", "C") for r in route])
    nA1 = np.cumsum([r == "A1" for r in route])
    nB = np.cumsum([r == "B" for r in route])
    act_list = [t for t in range(T) if route[t] in ("A1", "C")]
    posACT = {t: k for k, t in enumerate(act_list)}  # 0-based among ACT steps

    nc = bass.Bass("TRN2", target_bir_lowering=False, debug=False,
                   num_devices=N_CORES)
    qT = nc.dram_tensor("qT", [D, B], mybir.dt.float8e4,
                        kind="ExternalInput").ap()
    bankT = nc.dram_tensor("bankT", [D, N_SHARD], mybir.dt.float8e4,
                           kind="ExternalInput").ap()
    out = nc.dram_tensor("blockmax", [B, NFMAX], mybir.dt.float8e4,
                         kind="ExternalOutput").ap()
    raw = nc.dram_tensor("rawsim", [B, N_RAW * GROUP], mybir.dt.float8e4,
                         kind="ExternalOutput").ap()

    MAX = mybir.AluOpType.max

    with contextlib.ExitStack() as ctx:
        qs = ctx.enter_context(nc.sbuf_tensor([D, B], mybir.dt.float8e4))
        banks = ctx.enter_context(
            nc.sbuf_tensor([D, BANK_RING * 2 * GROUP], mybir.dt.float8e4))
        psum = ctx.enter_context(
            nc.psum_tensor([128, PSUM_RING * GROUP], mybir.dt.float32))
        stage = ctx.enter_context(
            nc.sbuf_tensor([128, STAGE_RING * GROUP], mybir.dt.float8e4))
        l1 = ctx.enter_context(
            nc.sbuf_tensor([128, GROUP // 2], mybir.dt.bfloat16))
        l2 = ctx.enter_context(
            nc.sbuf_tensor([128, GROUP // 4], mybir.dt.bfloat16))
        obuf = ctx.enter_context(
            nc.sbuf_tensor([128, 2 * NFMAX], mybir.dt.float8e4))
        # order-robust DMA tracking: one sem per bank ring slot / out unit
        qsem = ctx.enter_context(nc.semaphore("qsem"))       # qT load, +16
        bank_sem = [ctx.enter_context(nc.semaphore(f"bank_sem{i}"))
                    for i in range(BANK_RING)]               # +16 per load
        osem = [ctx.enter_context(nc.semaphore(f"osem{i}"))
                for i in range(N_UNITS)]                     # +16 per repeat
        mm_sem = ctx.enter_context(nc.semaphore("mm_sem"))   # +1/step (PE)
        evacA = ctx.enter_context(nc.semaphore("evacA"))     # +1/ACT copy
        dvedone = ctx.enter_context(nc.semaphore("dvedone"))  # +1/DVE B
        dvefold = ctx.enter_context(nc.semaphore("dvefold"))  # +1/DVE A1
        warmsem = ctx.enter_context(nc.semaphore("warmsem"))  # ACT warm
        b0sem = ctx.enter_context(nc.semaphore("b0sem"))  # first half-load
        block = ctx.enter_context(nc.Block())

        def psl(t, w):
            s = (t % PSUM_RING) * GROUP
            return psum[:, s:s + w]

        def oslot(t):
            c, bg, r, w, ns, col0 = info[t]
            f0 = FSLOT[t % N_STEPS]
            return obuf[:, c * NFMAX + f0:c * NFMAX + f0 + ns]

        @block.sync
        def _(sync):
            # one load = two bank groups (2048 cols; rump load is
            # smaller). The very first load is split in two 1024 halves
            # so the first matmul starts ~0.5us earlier.
            n_loads = repeats * ((N_BG + 1) // 2)
            lpr = (N_BG + 1) // 2  # loads per repeat

            def load_bank(ld, half=None):
                bg = (ld % lpr) * 2
                base = (ld % BANK_RING) * 2 * GROUP
                w = min(2 * GROUP, N_SHARD - bg * GROUP)
                sem = bank_sem[ld % BANK_RING]
                if half is not None:  # ld == 0 split
                    bg += half
                    base += half * GROUP
                    w = GROUP
                    if half == 0:
                        sem = b0sem
                sync.dma_start(
                    banks[:, base:base + w],
                    bankT[:, bg * GROUP:bg * GROUP + w],
                ).then_inc(sem, 16)

            load_bank(0, half=0)
            load_bank(0, half=1)
            for ld in range(1, min(BANK_RING, n_loads)):
                load_bank(ld)
            for ld in range(BANK_RING, n_loads):
                sync.wait_ge(mm_sem, 4 * (ld - BANK_RING) + 4)
                load_bank(ld)
            # final merged unit of each chunk per repeat: issue from the
            # (by now idle) SP stream, overlapping Pool's last raw DMA
            for r in range(repeats):
                for uo, (c, lo, hi, last_i) in enumerate(_OUT_UNITS):
                    if last_i < N_STEPS - 2:
                        continue
                    t = r * N_STEPS + last_i
                    sync.wait_ge(dvedone, nB[t])
                    sync.dma_start(
                        out[c * 128:(c + 1) * 128, lo:hi],
                        obuf[:, c * NFMAX + lo:c * NFMAX + hi],
                    ).then_inc(osem[uo], 16)

        @block.gpsimd
        def _(gpsimd):
            # ALL DMAs other than inputs go via SWDGE on the otherwise-
            # idle Pool engine: issuing a DMA serializes with the issuing
            # engine's stream, so neither SP (bank prefetch) nor ACT
            # (psum drain) can afford them
            gpsimd.dma_start(qs[:], qT).then_inc(qsem, 16)
            for t in range(T):
                i = t % N_STEPS
                if route[t] == "C":  # raw stage slice out
                    gpsimd.wait_ge(evacA, posACT[t] + 1)
                    c = info[t][0]
                    ss = (posACT[t] % STAGE_RING) * GROUP
                    rs = _RSLOT[i] * GROUP
                    gpsimd.dma_start(
                        raw[c * 128:(c + 1) * 128, rs:rs + GROUP],
                        stage[:, ss:ss + GROUP],
                    ).then_inc(osem[_unit_of(i)], 16)
                for uo, (c, lo, hi, last_i) in enumerate(_OUT_UNITS):
                    if i != last_i or last_i >= N_STEPS - 2:
                        continue
                    gpsimd.wait_ge(dvedone, nB[t])
                    if nA1[t]:
                        gpsimd.wait_ge(dvefold, nA1[t])
                    gpsimd.dma_start(
                        out[c * 128:(c + 1) * 128, lo:hi],
                        obuf[:, c * NFMAX + lo:c * NFMAX + hi],
                    ).then_inc(osem[uo], 16)

        @block.tensor
        def _(tensor):
            for t in range(T):
                c, bg, r, w, ns, col0 = info[t]
                u = t // 2
                ld = t // 4
                if t == 0:
                    tensor.wait_ge(qsem, 16)
                if t == 0:
                    tensor.wait_ge(b0sem, 16)         # first 1024 half
                elif t == 2:
                    tensor.wait_ge(bank_sem[0], 16)   # second half
                elif t % 4 == 0:
                    tensor.wait_ge(bank_sem[ld % BANK_RING],
                                   16 * (ld // BANK_RING + 1))
                if t >= PSUM_RING:
                    tp = t - PSUM_RING
                    if route[tp] in ("A1", "C"):
                        tensor.wait_ge(evacA, nACT[tp])
                    else:
                        tensor.wait_ge(dvedone, nB[tp])
                s = (t % PSUM_RING) * GROUP
                bb = ((t // 4) % BANK_RING) * 2 * GROUP + (bg % 2) * GROUP
                nmm = (w + 511) // 512
                for k in range(nmm):
                    kw = min(512, w - k * 512)
                    mm = tensor.matmul(
                        psum[:, s + k * 512: s + k * 512 + kw],
                        lhsT=qs[:, c * 128:(c + 1) * 128],
                        rhs=banks[:, bb + k * 512: bb + k * 512 + kw],
                        start=True, stop=True)
                    if k == nmm - 1:
                        mm.then_inc(mm_sem, 1)

        @block.scalar
        def _(scalar):
            # dummy copy at t~0: pulls the ACT function-table load fully
            # into the pipeline-fill phase (seeded by DVE's memset, no
            # input-DMA dependency)
            scalar.wait_ge(warmsem, 1)
            scalar.copy(stage[:, 2:4], stage[:, :2])
            scalar.drain()
            for idx, t in enumerate(act_list):
                if idx >= STAGE_RING:
                    occ = act_list[idx - STAGE_RING]
                    if route[occ] == "A1":
                        scalar.wait_ge(dvefold, nA1[occ])
                    else:  # C: raw out-DMA of occ frees the slot
                        scalar.wait_ge(osem[_unit_of(occ % N_STEPS)],
                                       16 * (occ // N_STEPS + 1))
                scalar.wait_ge(mm_sem, t + 1)
                ss = (idx % STAGE_RING) * GROUP
                scalar.copy(stage[:, ss:ss + GROUP],
                            psl(t, GROUP)).then_inc(evacA, 1)

        @block.vector
        def _(vector):
            vector.memset(stage[:, :2], 0.0).then_inc(warmsem, 1)
            vector.drain()
            # B-reduces free PSUM slots and are ready at matmul time;
            # A1 folds are ready only after ACT's copy. Emit folds late
            # so reduces never queue behind them.
            dve_ops = sorted(
                (t for t in range(T) if route[t] != "C"),
                key=lambda t: t if route[t] == "B" else t + 3.2)
            for t in dve_ops:
                r = route[t]
                if t >= N_STEPS:
                    vector.wait_ge(osem[_unit_of(t % N_STEPS)],
                                   16 * (t // N_STEPS))
                if r == "B":
                    vector.wait_ge(mm_sem, t + 1)
                    vector.tensor_reduce(
                        out=oslot(t),
                        in_=psl(t, info[t][3]).rearrange(
                            "p (b w) -> p b w", w=BLK),
                        axis=mybir.AxisListType.X,
                        op=MAX,
                    ).then_inc(dvedone, 1)
                else:  # A1
                    vector.wait_ge(evacA, posACT[t] + 1)
                    ss = (posACT[t] % STAGE_RING) * GROUP
                    h1, h2 = GROUP // 2, GROUP // 4
                    vector.tensor_tensor(
                        out=l1[:], in0=stage[:, ss:ss + h1],
                        in1=stage[:, ss + h1:ss + GROUP], op=MAX)
                    vector.drain()
                    vector.tensor_tensor(
                        out=l2[:], in0=l1[:, :h2], in1=l1[:, h2:], op=MAX)
                    vector.drain()
                    vector.tensor_tensor(
                        out=oslot(t), in0=l2[:, :h2 // 2],
                        in1=l2[:, h2 // 2:], op=MAX).then_inc(dvefold, 1)

    return nc


def _get_nc(repeats=1):
    if repeats not in _NC_CACHE:
        _NC_CACHE[repeats] = _build_nc(repeats)
    return _NC_CACHE[repeats]


def _prep_in_maps(query_feature, feature_bank):
    qT = np.ascontiguousarray(
        query_feature.astype(np.float32).T).astype(FP8)  # [128, 256]
    fb = np.asarray(feature_bank, dtype=np.float32)
    in_maps = []
    for i in range(N_CORES):
        shard = fb[i * N_SHARD:(i + 1) * N_SHARD]
        bt = np.ascontiguousarray(shard.T).astype(FP8)  # [128, 25000]
        in_maps.append({"qT": qT, "bankT": bt})
    return in_maps


def _chunk_layout(c):
    """Local col idx [SLOTS, BLK] for chunk c's compacted slot order:
    folded groups (bg order, per-route block pattern), then C groups
    (raw-slot order, contiguous-8)."""
    cols = np.empty((SLOTS, BLK), dtype=np.int64)
    k = np.arange(BLK)
    off = 0
    for bg in range(N_BG):
        i = 2 * bg + c
        _, _, r, w, ns, col0 = _step_info(i)
        if r == "C":
            continue
        j = np.arange(ns)
        if r == "A1":
            blk = j[:, None] + SPG * k[None, :]
        else:
            blk = BLK * j[:, None] + k[None, :]
        cols[off:off + ns] = col0 + blk
        off += ns
    assert off == NF[c]
    for i in [i for i in _C_STEPS if i % 2 == c]:
        _, _, r, w, ns, col0 = _step_info(i)
        j = np.arange(ns)
        cols[off:off + ns] = col0 + BLK * j[:, None] + k[None, :]
        off += ns
    assert off == SLOTS
    return cols


def _core_blockmax(bmx, rawx, c):
    """One core's per-block values for chunk c in compacted slot order.

    bmx: [256, NFMAX] f32, rawx: [256, N_RAW*GROUP] f32 -> [128, SLOTS]
    """
    rows = slice(c * 128, (c + 1) * 128)
    nraw_c = len([i for i in _C_STEPS if i % 2 == c])
    rb = rawx[rows, :nraw_c * GROUP].reshape(128, nraw_c * SPG, BLK)
    return np.concatenate([bmx[rows, :NF[c]], rb.max(axis=2)], axis=1)


def _run_device(query_feature, feature_bank, repeats=1, in_maps=None):
    if in_maps is None:
        in_maps = _prep_in_maps(query_feature, feature_bank)
    nc = _get_nc(repeats)
    res = run_bass_kernel_spmd(nc, in_maps, list(range(N_CORES)))
    bm = np.empty((N_CORES, B, SLOTS), dtype=np.float32)
    for core in range(N_CORES):
        bmx = res.results[core]["blockmax"].astype(np.float32)
        rawx = res.results[core]["rawsim"].astype(np.float32)
        for c in (0, 1):
            bm[core, c * 128:(c + 1) * 128] = _core_blockmax(bmx, rawx, c)
    return bm, res


def _host_topk(bm, query_feature, feature_bank, nsel=640):
    """bm: [8, 256, SLOTS] f32 per-block bounds (compacted order).
    Returns top-K indices [B, K] into the full bank, matching f32 jax
    top_k semantics."""
    q = np.asarray(query_feature, dtype=np.float32)
    fb = np.ascontiguousarray(np.asarray(feature_bank, dtype=np.float32))
    nblk = N_CORES * SLOTS
    # global block id (8 contiguous bank rows) per compacted slot
    gblk = np.empty((2, nblk), dtype=np.int64)
    for ch in range(2):
        srows = _chunk_layout(ch)  # [SLOTS, BLK] local cols
        assert (srows[:, 0] % BLK == 0).all()
        assert (srows == srows[:, :1] + np.arange(BLK)).all()
        for cidx in range(N_CORES):
            gblk[ch, cidx * SLOTS:(cidx + 1) * SLOTS] = (
                srows[:, 0] + cidx * N_SHARD) // BLK
    fbB = fb.reshape(N_TOTAL // BLK, BLK, D)
    bm_flat = bm.transpose(1, 0, 2).reshape(B, nblk)

    # partial descending order of block bounds (top-M is plenty; fall
    # back to a full sort only for queries that outgrow it)
    M = min(8192, nblk)
    part = np.argpartition(-bm_flat, M - 1, axis=1)[:, :M]
    pv = np.take_along_axis(bm_flat, part, axis=1)
    o_loc = np.argsort(-pv, axis=1)
    order = np.take_along_axis(part, o_loc, axis=1)
    sel_sorted = np.take_along_axis(bm_flat, order, axis=1)

    topk_idx = np.empty((B, K), dtype=np.int64)
    pending = np.arange(B)
    nb = nsel
    while len(pending):
        if nb > M and M < nblk:
            order_f = np.argsort(-bm_flat[pending], axis=1)
            order = np.zeros((B, nblk), dtype=np.int64)
            order[pending] = order_f
            sel_sorted = np.full((B, nblk), -np.inf, dtype=np.float32)
            sel_sorted[pending] = np.take_along_axis(
                bm_flat[pending], order_f, axis=1)
            M = nblk
        nb = min(nb, nblk)
        P = len(pending)
        gids = gblk[(pending // 128)[:, None], order[pending, :nb]]
        gids.sort(axis=1)  # sorted gather is ~2x faster, order is free
        sims = np.einsum("qbrd,qd->qbr", fbB[gids], q[pending],
                         optimize=True).reshape(P, -1)
        rows = (gids[:, :, None] * BLK + np.arange(BLK)).reshape(P, -1)
        o = np.lexsort((rows, -sims), axis=-1)[:, :K]
        tK = sims[np.arange(P), o[:, -1]]
        unsel = (sel_sorted[pending, nb] if nb < nblk
                 else np.full(P, -np.inf, dtype=np.float32))
        done = (unsel + MARGIN < tK) | (nb >= nblk)
        sel = np.take_along_axis(rows, o, axis=1)
        topk_idx[pending[done]] = sel[done]
        pending = pending[~done]
        nb *= 2
    return topk_idx


def _labels_to_output(topk_idx, target_bank):
    tb = np.asarray(target_bank).astype(np.int64)
    lab = tb[topk_idx]  # [B, K]
    mask = np.zeros((B, NUM_CLASSES), dtype=bool)
    mask[np.arange(B)[:, None], lab] = True
    # inf vote weights -> membership only: voted classes (ascending) first,
    # then unvoted (ascending); matches stable argsort of -scores.
    return np.argsort(~mask, axis=1, kind="stable").astype(np.int32)


def kernel(query_feature, feature_bank, target_bank):
    query_feature = np.asarray(query_feature)
    feature_bank = np.asarray(feature_bank)
    target_bank = np.asarray(target_bank)
    bm, _ = _run_device(query_feature, feature_bank)
    topk_idx = _host_topk(bm, query_feature, feature_bank)
    return _labels_to_output(topk_idx, target_bank)
